# revision 17
# baseline (speedup 1.0000x reference)
"""Chunked causal self-attention with RoPE on 8 Trainium2 NeuronCores.

Problem: B=4, L=4096, H=16, DH=DV=128, CHUNK=1024 (N=4 chunks).
RoPE on q,k then chunk-local causal attention per (batch, chunk, head).

Sharding: heads split across 8 cores (2 heads/core) -> 32 independent
(1024 x 1024, d=128) attention problems per core, grouped 4-per-(b,h)
so one RoPE table pass covers a whole group.

v2 design notes (from trace analysis of v1):
  - All inputs packed d-major as (128, NPROB*1024) fp16 so every group
    load is one DMA with 128 x 8KB descriptors; a single sync-queue
    stream sustains ~400GB/s this way.
  - PE work: per problem 12 score matmuls (4608 cols), 12 PV matmuls
    (4608 cols), 2 denominator matmuls over an fp16 block-sum R (1024
    cols). No mask matmuls: causal diag masking is a DVE multiply with
    an upper-tri 0/1 tile on the fp16 P-tile.
  - Software pipeline one problem deep (PE order: PV(p-1), den(p-1),
    scores(p)) keeps the PE continuously busy so it reaches the 2.4GHz
    p-state instead of 1.2GHz.
  - exp on ScalarE; narrow blocks are paired into shared PSUM tiles so
    there are 6 activation instructions per problem instead of 8.
  - Normalization (outT/den) and final layout transposes on host.
"""

import math
import os
import sys

import numpy as np

for _p in ("/opt/trn_rl_repo", "/root/.axon_site/_ro/trn_rl_repo"):
    if os.path.isdir(_p) and _p not in sys.path:
        sys.path.insert(0, _p)

import concourse.bass as bass  # noqa: E402
import concourse.tile as tile  # noqa: E402
from concourse import bass_utils, mybir  # noqa: E402

B, L, H, DH, DV = 4, 4096, 16, 128, 128
CHUNK = 1024
NCHUNK = L // CHUNK  # 4
ROPE_BASE = 10000.0
NCORES = 8
HPC = H // NCORES  # 2 heads per core
NPROB = B * HPC * NCHUNK  # 32 problems per core
NG = B * HPC  # 8 groups of 4 chunks
HALF = DH // 2  # 64
NB = CHUNK // 128  # 8 k-blocks
SCALE = 1.0 / math.sqrt(DH)

F16 = mybir.dt.float16
F32 = mybir.dt.float32
AF = mybir.ActivationFunctionType

# exp "units": list of (psS column offset, block) pairs per unit.
# Blocks 0-3 get their own [128,1024] PSUM tile; (4,5) and (6,7) share.
# Each entry: (unit_tag, [(block, tile_col_off)])
UNITS = [
    ("u0", [(0, 0)]),
    ("u1", [(1, 128)]),
    ("u2", [(2, 256)]),
    ("u3", [(3, 384)]),
    ("u45", [(4, 0), (5, 512)]),
    ("u67", [(6, 0), (7, 256)]),
]
# For single blocks the tile col offset equals q0 so piece splits at 512
# stay bank-aligned. For paired units each block's region starts at a
# bank boundary (0 / 512) or stays within one bank.


def _block_region(b):
    """absolute q range covered for k-block b (causal)."""
    return 128 * b, CHUNK


def build_module(nprob=NPROB):
    from concourse import bacc

    nc = bacc.Bacc("TRN2", target_bir_lowering=False, debug=False)

    qT = nc.dram_tensor("qT_in", (128, nprob * CHUNK), F16, kind="ExternalInput")
    kT = nc.dram_tensor("kT_in", (128, nprob * CHUNK), F16, kind="ExternalInput")
    vT = nc.dram_tensor("vT_in", (128, nprob * CHUNK), F16, kind="ExternalInput")
    c2 = nc.dram_tensor("c2_in", (128, L), F16, kind="ExternalInput")
    s2 = nc.dram_tensor("s2_in", (128, L), F16, kind="ExternalInput")
    tri = nc.dram_tensor("tri_in", (128, 128), F16, kind="ExternalInput")
    ones = nc.dram_tensor("ones_in", (128, 1), F16, kind="ExternalInput")

    outT = nc.dram_tensor("outT_out", (128, nprob * CHUNK), F16, kind="ExternalOutput")
    den = nc.dram_tensor(
        "den_out", (nprob // NCHUNK, NCHUNK * CHUNK), F16, kind="ExternalOutput"
    )

    with tile.TileContext(nc) as tc:
        _body(tc, nprob, qT, kT, vT, c2, s2, tri, ones, outT, den)
    nc.compile()
    return nc


def _body(tc, nprob, qT, kT, vT, c2, s2, tri, ones, outT, den):
    from contextlib import ExitStack

    nc = tc.nc
    ngroups = nprob // NCHUNK
    GW = NCHUNK * CHUNK  # group width: 4096 cols

    with ExitStack() as ctx:
        consts = ctx.enter_context(tc.tile_pool(name="consts", bufs=1))
        ing = ctx.enter_context(tc.tile_pool(name="ing", bufs=2))
        qkp = ctx.enter_context(tc.tile_pool(name="qkp", bufs=2))
        ptp = ctx.enter_context(tc.tile_pool(name="ptp", bufs=16))
        rp = ctx.enter_context(tc.tile_pool(name="rp", bufs=2))
        outp = ctx.enter_context(tc.tile_pool(name="outp", bufs=2))
        dnp = ctx.enter_context(tc.tile_pool(name="dnp", bufs=2))
        psSp = ctx.enter_context(tc.tile_pool(name="psS", bufs=3, space="PSUM"))
        psOp = ctx.enter_context(tc.tile_pool(name="psO", bufs=1, space="PSUM"))

        c2_t = consts.tile([128, L], F16, tag="c2")
        nc.sync.dma_start(out=c2_t, in_=c2.ap())
        s2_t = consts.tile([128, L], F16, tag="s2")
        nc.sync.dma_start(out=s2_t, in_=s2.ap())
        tri_t = consts.tile([128, 128], F16, tag="tri")
        nc.sync.dma_start(out=tri_t, in_=tri.ap())
        ones_t = consts.tile([128, 1], F16, tag="ones")
        nc.sync.dma_start(out=ones_t, in_=ones.ap())

        # touch consts once so compute ops don't carry extra DMA waits
        dummy = consts.tile([128, 1], F16, tag="dummy")
        nc.vector.tensor_copy(out=dummy, in_=c2_t[:, 0:1])
        nc.vector.tensor_copy(out=dummy, in_=s2_t[:, 0:1])
        nc.vector.tensor_copy(out=dummy, in_=tri_t[:, 0:1])
        nc.vector.tensor_copy(out=dummy, in_=ones_t)

        state = {}  # per live problem: pt tiles, R, group tiles

        def emit_loads_rope(g):
            qg = ing.tile([128, GW], F16, tag="qg")
            nc.sync.dma_start(out=qg, in_=qT.ap()[:, g * GW:(g + 1) * GW])
            kg = ing.tile([128, GW], F16, tag="kg")
            nc.sync.dma_start(out=kg, in_=kT.ap()[:, g * GW:(g + 1) * GW])
            vg = ing.tile([128, GW], F16, tag="vg")
            nc.sync.dma_start(out=vg, in_=vT.ap()[:, g * GW:(g + 1) * GW])
            # swapped-half copies [x2; x1] via SBUF->SBUF DMA (partition
            # shift is illegal for 2-input DVE ops but fine for DMA)
            qsw = ing.tile([128, GW], F16, tag="qsw")
            nc.sync.dma_start(out=qsw[0:HALF, :], in_=qg[HALF:128, :])
            nc.sync.dma_start(out=qsw[HALF:128, :], in_=qg[0:HALF, :])
            ksw = ing.tile([128, GW], F16, tag="ksw")
            nc.sync.dma_start(out=ksw[0:HALF, :], in_=kg[HALF:128, :])
            nc.sync.dma_start(out=ksw[HALF:128, :], in_=kg[0:HALF, :])

            def rope(src, swp, tag):
                # src *= [c;c]; swp *= [-s;s]; dst = src + swp  (in place)
                nc.vector.tensor_mul(src, src, c2_t)
                nc.vector.tensor_mul(swp, swp, s2_t)
                dst = qkp.tile([128, GW], F16, name=tag, tag=tag)
                nc.vector.tensor_add(dst, src, swp)
                return dst

            qp = rope(qg, qsw, "qp")
            kp = rope(kg, ksw, "kp")
            state[("grp", g)] = (vg, qp, kp)

        def emit_scores_exp(p):
            g, pi = divmod(p, NCHUNK)
            vg, qp, kp = state[("grp", g)]
            poff = pi * CHUNK
            pts = {}
            for tag, blocks in UNITS:
                ps = psSp.tile([128, CHUNK], F32, tag="psS")
                # score matmuls for each block in this unit
                for b, off in blocks:
                    q0, q1 = _block_region(b)
                    kblk = kp[:, poff + 128 * b: poff + 128 * (b + 1)]
                    # pieces of [q0,q1) split at bank boundaries rel. tile
                    # tile col of abs q is (q - q0 + off)
                    a = q0
                    while a < q1:
                        # bank boundary in tile coords
                        ta = a - q0 + off
                        bank_end = ((ta // 512) + 1) * 512
                        e = min(q1, a + (bank_end - ta))
                        nc.tensor.matmul(
                            ps[:, ta:ta + (e - a)],
                            lhsT=kblk,
                            rhs=qp[:, poff + a: poff + e],
                            start=True, stop=True,
                        )
                        a = e
                pt = ptp.tile([128, CHUNK], F16, tag="pt")
                # one exp over the full used span of this unit tile
                lo_off = min(off for b, off in blocks)
                hi_off = max(off + (CHUNK - 128 * b) for b, off in blocks)
                nc.scalar.activation(
                    out=pt[:, lo_off:hi_off], in_=ps[:, lo_off:hi_off],
                    func=AF.Exp, scale=SCALE,
                )
                for b, off in blocks:
                    pts[b] = (pt, off)
            state[("pt", p)] = pts

        def emit_masks_r(p):
            pts = state[("pt", p)]
            # diag masks in place: blocks 0-3 on DVE, 4-7 on GpSimd
            for b in range(NB):
                ptb, offb = pts[b]
                eng = nc.vector if b < 4 else nc.gpsimd
                eng.tensor_mul(
                    ptb[:, offb:offb + 128], ptb[:, offb:offb + 128], tri_t
                )
            # Ra = sum of blocks 0-3 (DVE); Rb = sum of blocks 4-7 (GpSimd,
            # covers abs q [512,1024) only). den later = ones@Ra + ones@Rb.
            Ra = rp.tile([128, CHUNK], F16, name="Ra", tag="Ra")
            pt0, off0 = pts[0]
            nc.vector.tensor_copy(out=Ra, in_=pt0[:, off0:off0 + CHUNK])
            for b in range(1, 4):
                ptb, offb = pts[b]
                q0, q1 = _block_region(b)
                nc.vector.tensor_add(
                    Ra[:, q0:q1], Ra[:, q0:q1], ptb[:, offb:offb + (q1 - q0)]
                )
            Rb = rp.tile([128, 512], F16, name="Rb", tag="Rb")
            pt4, off4 = pts[4]
            nc.gpsimd.tensor_copy(out=Rb, in_=pt4[:, off4:off4 + 512])
            for b in range(5, NB):
                ptb, offb = pts[b]
                q0, q1 = _block_region(b)
                nc.gpsimd.tensor_add(
                    Rb[:, q0 - 512:512], Rb[:, q0 - 512:512],
                    ptb[:, offb:offb + (q1 - q0)]
                )
            state[("R", p)] = (Ra, Rb)

        def emit_pv_den(p):
            g, pi = divmod(p, NCHUNK)
            vg, qp, kp = state[("grp", g)]
            poff = pi * CHUNK
            pts = state.pop(("pt", p))
            R = state.pop(("R", p))
            pso = psOp.tile([128, CHUNK], F32, tag="psO")
            # accumulate over blocks; per psum bank the last writer stops
            last_in_bank = {0: 3, 1: NB - 1}  # bank0 cols [0,512): blocks 0..3
            for b in range(NB):
                q0, q1 = _block_region(b)
                ptb, offb = pts[b]
                vblk = vg[:, poff + 128 * b: poff + 128 * (b + 1)]
                a = q0
                while a < q1:
                    bank = a // 512
                    e = min(q1, (bank + 1) * 512)
                    nc.tensor.matmul(
                        pso[:, a:e],
                        lhsT=vblk,
                        rhs=ptb[:, offb + (a - q0): offb + (e - q0)],
                        start=(b == 0),
                        stop=(b == last_in_bank[bank]),
                    )
                    a = e
            Ra, Rb = R
            psd = psSp.tile([128, CHUNK], F32, name="psd", tag="psS")
            nc.tensor.matmul(psd[0:1, 0:512], lhsT=ones_t, rhs=Ra[:, 0:512],
                             start=True, stop=True)
            nc.tensor.matmul(psd[0:1, 512:CHUNK], lhsT=ones_t, rhs=Ra[:, 512:CHUNK],
                             start=True, stop=False)
            nc.tensor.matmul(psd[0:1, 512:CHUNK], lhsT=ones_t, rhs=Rb,
                             start=False, stop=True)

            # drain psO (GpSimd) / psd (DVE) to SBUF fp16, batched per group
            if pi == 0:
                state[("outg", g)] = outp.tile([128, GW], F16, name="outg", tag="outg")
                state[("deng", g)] = dnp.tile([1, GW], F16, name="deng", tag="deng")
            outg = state[("outg", g)]
            deng = state[("deng", g)]
            nc.vector.tensor_copy(out=outg[:, poff:poff + CHUNK], in_=pso)
            nc.vector.tensor_copy(out=deng[:, poff:poff + CHUNK], in_=psd[0:1, :])
            if pi == NCHUNK - 1:
                nc.sync.dma_start(
                    out=outT.ap()[:, g * GW:(g + 1) * GW], in_=outg
                )
                nc.sync.dma_start(out=den.ap()[g], in_=deng)
                state.pop(("outg", g))
                state.pop(("deng", g))
                state.pop(("grp", g))

        # main software-pipelined loop; per step: scores(p) first on PE so
        # it never waits on exp(p-1), then PV/den(p-1).
        for p in range(nprob + 1):
            if p < nprob:
                if p == 0:
                    emit_loads_rope(0)
                if p % NCHUNK == 1 and (p // NCHUNK) + 1 < ngroups:
                    emit_loads_rope(p // NCHUNK + 1)
                emit_scores_exp(p)
            if p > 0:
                emit_pv_den(p - 1)
            if p < nprob:
                emit_masks_r(p)


def _host_consts():
    freqs = np.exp(np.arange(HALF, dtype=np.float64) * (-math.log(ROPE_BASE) / HALF))
    pos = np.arange(L, dtype=np.float64)
    ang = pos[:, None] * freqs[None, :]  # (L, 64)
    cos = np.cos(ang)
    sin = np.sin(ang)
    # C2 = [c;c]; with qsw = [q2;q1] the S table is [-s;+s]:
    #   q'[0:64]  = q1*c + q2*(-s);  q'[64:128] = q2*c + q1*(+s)
    c2 = np.concatenate([cos.T, cos.T], axis=0).astype(np.float16)  # (128, L)
    s2 = np.concatenate([-sin.T, sin.T], axis=0).astype(np.float16)
    r = np.arange(128)
    tri = (r[None, :] >= r[:, None]).astype(np.float16)  # keep q >= k
    ones = np.ones((128, 1), np.float16)
    return c2, s2, tri, ones


def _pack_core(qc, kc, vc):
    """qc,kc,vc: (B, L, HPC, 128) fp32 -> d-major (128, NPROB*1024) fp16."""

    def dmaj(x):
        # (B, L, h, D) -> (b, h, n, j, d) -> (d, b, h, n, j)
        a = x.transpose(0, 2, 1, 3).reshape(B, HPC, NCHUNK, CHUNK, DH)
        a = a.transpose(4, 0, 1, 2, 3).reshape(DH, NPROB * CHUNK)
        return np.ascontiguousarray(a).astype(np.float16)

    # v: partition = k-within-block, cols = (b,h,n, block, dv)
    a = vc.transpose(0, 2, 1, 3).reshape(B, HPC, NCHUNK, NB, 128, DV)
    a = a.transpose(4, 0, 1, 2, 3, 5).reshape(128, NPROB * CHUNK)
    vp = np.ascontiguousarray(a).astype(np.float16)
    return dict(qT_in=dmaj(qc), kT_in=dmaj(kc), vT_in=vp)


_NC_CACHE = {}
LAST_RESULT = None


def _get_module(nprob=NPROB):
    if nprob not in _NC_CACHE:
        _NC_CACHE[nprob] = build_module(nprob)
    return _NC_CACHE[nprob]


def kernel(q, k, v):
    q = np.asarray(q, dtype=np.float32)
    k = np.asarray(k, dtype=np.float32)
    v = np.asarray(v, dtype=np.float32)

    c2, s2, tri, ones = _host_consts()
    consts = dict(c2_in=c2, s2_in=s2, tri_in=tri, ones_in=ones)

    in_maps = []
    for c in range(NCORES):
        hs = slice(HPC * c, HPC * (c + 1))
        m = _pack_core(q[:, :, hs], k[:, :, hs], v[:, :, hs])
        m.update(consts)
        in_maps.append(m)

    nc = _get_module(NPROB)
    trace = bool(int(os.environ.get("KERNEL_TRACE", "0")))
    res = bass_utils.run_bass_kernel_spmd(
        nc, in_maps, core_ids=list(range(NCORES)), trace=trace
    )
    global LAST_RESULT
    LAST_RESULT = res

    out = np.empty((B, L, H, DV), np.float32)
    for c in range(NCORES):
        ot = res.results[c]["outT_out"].astype(np.float32)  # (128dv, 32*1024)
        dn = res.results[c]["den_out"].astype(np.float32).reshape(-1)  # (32*1024,)
        o = ot / dn[None, :]
        # (dv, b, h, n, j) -> (b, n*j=L, h, dv)
        o = o.reshape(DV, B, HPC, NCHUNK, CHUNK).transpose(1, 3, 4, 2, 0)
        out[:, :, HPC * c:HPC * (c + 1)] = o.reshape(B, L, HPC, DV)
    return out
